# revision 1
# baseline (speedup 1.0000x reference)
"""Contrastive-loss kernel for Trainium2, 8 NeuronCores.

Math
----
reference:
    yn  = ys / clip(||ys||, 1e-6)         (row-normalize)
    cos = yn @ yn.T                        [B, B]
    pair_loss = same ? relu(2 - cos)^2 : cos^2
    loss = sum(strict_lower(pair_loss)) / (B*(B-1)/2)

Because margin M = 2 and |cos| <= 1, relu(2 - cos) == 2 - cos always, so
    pair_loss = cos^2 + 4 * same * (1 - cos)
and since cos / same are symmetric with cos_ii == 1, same_ii == 1:
    sum_{i>j} pair_loss = (F1 - B) / 2 + 2 * F2
where over the FULL matrix
    F1 = sum_ij cos_ij^2
    F2 = sum_ij same_ij * (1 - cos_ij) = sum_ij same_ij - sum_ij same_ij*cos_ij

No triangle masking and no relu are needed: each core computes its
512-row block of the full Gram matrix and three per-partition sums
(sum cos^2, sum same, sum same*cos); the host combines 8x[128] partials.

Device plan (SPMD, identical program on 8 cores; only input data differs):
 1. Each core loads its own 512 rows (f32), computes row norms
    (ACT Square+accum), normalizes, casts to bf16, transposes its
    [512, 2048] shard to K-major [2048, 512] via PE transpose.
 2. AllGather of the bf16 transposed shards -> full ynT [C*2048, 512-blocks].
 3. bf16 Gram matmul: lhsT = own ynT columns, rhs = gathered ynT
    (SBUF-resident), accumulated over K in PSUM (f32).
 4. Epilogue per [128, 512] tile: ACT Square+accum (sum cos^2),
    DVE is_equal+accum (sum same), DVE tensor_tensor_reduce (sum same*cos).
 5. DMA out [128, 4] f32 partials; host reduces.
"""

import os
import sys

for _p in ("/opt/trn_rl_repo", "/root/.axon_site/_ro/trn_rl_repo"):
    if _p not in sys.path and os.path.isdir(_p):
        sys.path.append(_p)

import numpy as np

import concourse.bass as bass
import concourse.mybir as mybir
import concourse.tile as tile
from concourse import masks
from concourse.bass import ds, ts  # noqa: F401

F32 = mybir.dt.float32
BF16 = mybir.dt.bfloat16
AF = mybir.ActivationFunctionType
ALU = mybir.AluOpType

P = 128  # partitions


def _split_multi_waits(nc):
    """Split instructions carrying >1 semaphore wait.

    The walrus in this environment rejects compute instructions with more
    than one sync-wait command ("Too many sync wait commands"). Move the
    extra waits onto standalone EventSemaphore instructions inserted just
    before, on the same engine — semantically identical (the engine's
    sequencer blocks on each in order).
    """
    n_split = 0
    for fn in nc.m.functions:
        for bb in fn.blocks:
            new_insts = []
            for ins in bb.instructions:
                si = ins.sync_info
                if (
                    si is not None
                    and len(si.on_wait) > 1
                    and not isinstance(ins, mybir.InstEventSemaphore)
                ):
                    extra = list(si.on_wait[1:])
                    ins.sync_info = mybir.SyncInfo(
                        on_wait=[si.on_wait[0]], on_update=list(si.on_update)
                    )
                    for w in extra:
                        n_split += 1
                        ev = mybir.InstEventSemaphore(
                            name=f"antsplitwait_{n_split}_{ins.name}",
                            engine=ins.engine,
                            ins=[],
                            outs=[],
                            sync_info=mybir.SyncInfo(on_wait=[w], on_update=[]),
                            bass_nofuse=True,
                        )
                        new_insts.append(ev)
                new_insts.append(ins)
            bb.instructions = new_insts
    return n_split


def build_gram_loss(B=4096, D=2048, C=8, NT=512, S=4):
    """Build the SPMD bass program (one nc, run on C cores).

    B: total rows; D: features; C: cores; NT: N tile of the Gram matmul.
    S: AllGather split factor — the shard is gathered in S column-chunks so
    the Gram matmul can start after the first chunk and overlap the rest.
    Gathered chunk s holds global rows {r*Bs + s*W .. +W} for r in 0..C-1,
    contiguous in SBUF — labels must be host-permuted to match (see
    make_in_maps / column_perm).
    """
    assert B % (C * P) == 0 and D % P == 0 and B % NT == 0
    Bs = B // C          # rows per core
    RT = Bs // P         # 128-row tiles per core
    KC = D // P          # K chunks
    NJ = B // NT         # N tiles over all columns
    assert Bs % S == 0
    W = Bs // S          # chunk width per core
    assert (C * W) % NT == 0 or NT % (C * W) == 0

    nc = bass.Bass(num_devices=C)

    ys_mine = nc.dram_tensor("ys_mine", [Bs, D], F32, kind="ExternalInput")
    labels_all = nc.dram_tensor("labels_all", [1, B], F32, kind="ExternalInput")
    labels_mine = nc.dram_tensor("labels_mine", [RT, P], F32, kind="ExternalInput")
    out_parts = nc.dram_tensor("out_parts", [P, 4], F32, kind="ExternalOutput")

    # Shared scratchpad output is the fast path but only supported for >4 cores
    cc_space = "Shared" if C > 4 else "Local"
    cc_ins = [nc.dram_tensor(f"cc_in{s}", [D, W], BF16) for s in range(S)]
    cc_outs = [
        nc.dram_tensor(f"cc_out{s}", [C * D, W], BF16, addr_space=cc_space)
        for s in range(S)
    ]

    with tile.TileContext(nc) as tc:
        with (
            tc.tile_pool(name="const", bufs=1) as const_pool,
            tc.tile_pool(name="big", bufs=1) as big_pool,
            tc.tile_pool(name="ysin", bufs=2) as ys_pool,
            tc.tile_pool(name="yn", bufs=2) as yn_pool,
            tc.tile_pool(name="sqscr", bufs=1) as sq_scratch_pool,
            tc.tile_pool(name="small", bufs=4) as small_pool,
            tc.tile_pool(name="acc", bufs=1) as acc_pool,
            tc.tile_pool(name="ep", bufs=3) as ep_pool,
            tc.tile_pool(name="red", bufs=6) as red_pool,
            tc.tile_pool(name="pt", bufs=2, space="PSUM") as pt_psum,
            tc.tile_pool(name="mm", bufs=5, space="PSUM") as mm_psum,
            tc.tile_pool(name="lab", bufs=1, space="PSUM") as lab_psum,
        ):
            # ---------------- constants / label prep ----------------
            identity = const_pool.tile([P, P], BF16)
            masks.make_identity(nc, identity[:])

            ones_1xP = const_pool.tile([1, P], BF16)
            nc.gpsimd.memset(ones_1xP[:], 1.0)

            eps_tile = const_pool.tile([P, 1], F32)
            nc.gpsimd.memset(eps_tile[:], 1e-6)

            # own labels, per-partition: [P, RT] f32 (tensor_scalar is_equal
            # requires an f32 scalar operand; values 0..9 are exact)
            l_mine = const_pool.tile([P, RT], F32)
            nc.gpsimd.dma_start(
                out=l_mine[:], in_=labels_mine[:, :].rearrange("t p -> p t")
            )

            # all labels on one partition, bf16
            lab_row = const_pool.tile([1, B], BF16)
            nc.gpsimd.dma_start(out=lab_row[:], in_=labels_all[:, :])

            # broadcast labels across partitions: L_col[p, j] = label[j]
            L_col = big_pool.tile([P, B], BF16)
            for jb in range(B // NT):
                ps_lab = lab_psum.tile([P, NT], F32)
                nc.tensor.matmul(
                    ps_lab[:],
                    lhsT=ones_1xP[:],
                    rhs=lab_row[:, ts(jb, NT)],
                    start=True,
                    stop=True,
                )
                nc.scalar.copy(L_col[:, ts(jb, NT)], ps_lab[:])

            # accumulators
            acc_sq = acc_pool.tile([P, 1], F32)
            acc_eq = acc_pool.tile([P, 1], F32)
            acc_eqc = acc_pool.tile([P, 1], F32)
            nc.vector.memset(acc_sq[:], 0.0)
            nc.vector.memset(acc_eq[:], 0.0)
            nc.vector.memset(acc_eqc[:], 0.0)

            # ---------------- phase A: normalize + transpose own shard ----
            ynT_mine = big_pool.tile([P, KC, Bs], BF16)

            for t in range(RT):
                ys_t = ys_pool.tile([P, D], F32)
                # alternate HWDGE rings (SP / ACT) so big DMAs don't
                # serialize on one FIFO
                dmae = nc.sync if t % 2 == 0 else nc.scalar
                dmae.dma_start(out=ys_t[:], in_=ys_mine[ts(t, P), :])

                yn_t = yn_pool.tile([P, D], BF16)
                ssq = small_pool.tile([P, 1], F32)
                # dedicated scratch for the squares: aliasing yn_t here adds a
                # cross-engine WAW dep that overflows the ISA sync-wait slots
                sq_scratch = sq_scratch_pool.tile([P, D], BF16)
                nc.scalar.activation(
                    sq_scratch[:], ys_t[:], AF.Square, accum_out=ssq[:]
                )
                norm_t = small_pool.tile([P, 1], F32)
                nc.scalar.sqrt(norm_t[:], ssq[:])
                normc = small_pool.tile([P, 1], F32)
                # max against a memset tile: a float immediate here lowers to
                # a const-AP read whose extra dep overflows ISA sync-wait slots
                nc.vector.tensor_tensor(
                    normc[:], norm_t[:], eps_tile[:], ALU.max
                )
                r_t = small_pool.tile([P, 1], F32)
                nc.vector.reciprocal(r_t[:], normc[:])
                nc.vector.tensor_scalar_mul(yn_t[:], ys_t[:], r_t[:])

                for kc in range(KC):
                    pt = pt_psum.tile([P, P], BF16)
                    nc.tensor.transpose(pt[:], yn_t[:, ts(kc, P)], identity[:])
                    ev = nc.scalar if kc % 2 == 0 else nc.vector
                    if ev is nc.scalar:
                        nc.scalar.copy(ynT_mine[:, kc, ts(t, P)], pt[:])
                    else:
                        nc.vector.tensor_copy(ynT_mine[:, kc, ts(t, P)], pt[:])

            # ship shard chunks to DRAM and gather them one chunk at a time;
            # the matmul starts once chunk 0 is in SBUF
            rhs_all = big_pool.tile([P, KC, B], BF16)
            for s in range(S):
                nc.sync.dma_start(
                    out=cc_ins[s][:, :].rearrange("(kc p) c -> p kc c", p=P),
                    in_=ynT_mine[:, :, ts(s, W)],
                )
            for s in range(S):
                nc.gpsimd.collective_compute(
                    "AllGather",
                    ALU.bypass,
                    replica_groups=[list(range(C))],
                    ins=[cc_ins[s][:, :]],
                    outs=[cc_outs[s][:, :]],
                )
                for r in range(C):
                    dmae = nc.sync if r % 2 == 0 else nc.scalar
                    dmae.dma_start(
                        out=rhs_all[:, :, s * C * W + r * W : s * C * W + (r + 1) * W],
                        in_=cc_outs[s][r * D : (r + 1) * D, :].rearrange(
                            "(kc p) c -> p kc c", p=P
                        ),
                    )

            # ---------------- phase C: Gram + epilogue ----------------
            # j outer: all row-tiles of an AllGather chunk's columns run
            # before any tile that needs a later chunk — the PE never
            # stalls on a not-yet-gathered chunk while ready work exists
            for j in range(NJ):
                for i in range(RT):
                    ps = mm_psum.tile([P, NT], F32)
                    for kc in range(KC):
                        nc.tensor.matmul(
                            ps[:],
                            lhsT=ynT_mine[:, kc, ts(i, P)],
                            rhs=rhs_all[:, kc, ts(j, NT)],
                            start=(kc == 0),
                            stop=(kc == KC - 1),
                        )

                    # S1 += sum cos^2
                    sq_scr = ep_pool.tile([P, NT], BF16, tag="sq")
                    sq_red = red_pool.tile([P, 1], F32, tag="sqr")
                    nc.scalar.activation(
                        sq_scr[:], ps[:], AF.Square, accum_out=sq_red[:]
                    )
                    nc.vector.tensor_tensor(
                        acc_sq[:], acc_sq[:], sq_red[:], ALU.add
                    )

                    # SB += sum same
                    eq_t = ep_pool.tile([P, NT], BF16, tag="eq")
                    eq_red = red_pool.tile([P, 1], F32, tag="eqr")
                    nc.vector.tensor_scalar(
                        eq_t[:],
                        L_col[:, ts(j, NT)],
                        l_mine[:, i : i + 1],
                        None,
                        ALU.is_equal,
                        op1=ALU.add,
                        accum_out=eq_red[:],
                    )
                    nc.vector.tensor_tensor(
                        acc_eq[:], acc_eq[:], eq_red[:], ALU.add
                    )

                    # SC += sum same * cos
                    # (tensor_tensor_reduce lowers to a raw-ISA op this
                    # walrus rejects — use mult + tensor_reduce instead)
                    eqc_scr = ep_pool.tile([P, NT], F32, tag="eqc")
                    eqc_red = red_pool.tile([P, 1], F32, tag="eqcr")
                    nc.vector.tensor_tensor(
                        eqc_scr[:], eq_t[:], ps[:], ALU.mult
                    )
                    nc.vector.tensor_reduce(
                        eqc_red[:], eqc_scr[:], mybir.AxisListType.X, ALU.add
                    )
                    nc.vector.tensor_tensor(
                        acc_eqc[:], acc_eqc[:], eqc_red[:], ALU.add
                    )

            # ---------------- phase D: write partials ----------------
            out_sb = const_pool.tile([P, 4], F32)
            nc.vector.memset(out_sb[:], 0.0)
            nc.scalar.copy(out_sb[:, 0:1], acc_sq[:])
            nc.scalar.copy(out_sb[:, 1:2], acc_eq[:])
            nc.scalar.copy(out_sb[:, 2:3], acc_eqc[:])
            nc.sync.dma_start(out=out_parts[:, :], in_=out_sb[:])

    _split_multi_waits(nc)
    return nc


def column_perm(B, C, S):
    """Global row index held at each SBUF rhs column (see build_gram_loss)."""
    Bs = B // C
    W = Bs // S
    idx = np.arange(B)
    s, r, c = idx // (C * W), (idx // W) % C, idx % W
    return r * Bs + s * W + c


def make_in_maps(ys, labels, B, D, C, S=4):
    """Shard host inputs into per-core input maps."""
    ys = np.ascontiguousarray(ys, dtype=np.float32)
    lab_f = labels.astype(np.float32)
    Bs = B // C
    RT = Bs // P
    lab_all = lab_f[column_perm(B, C, S)].reshape(1, B)
    in_maps = []
    for k in range(C):
        in_maps.append(
            {
                "ys_mine": ys[k * Bs : (k + 1) * Bs],
                "labels_all": lab_all,
                "labels_mine": lab_f[k * Bs : (k + 1) * Bs].reshape(RT, P),
            }
        )
    return in_maps


def combine_parts(parts_list, B):
    """parts_list: per-core [128, 4] f32 partials -> scalar loss."""
    s1 = 0.0
    sb = 0.0
    sc = 0.0
    for p in parts_list:
        p = np.asarray(p, dtype=np.float64)
        s1 += p[:, 0].sum()
        sb += p[:, 1].sum()
        sc += p[:, 2].sum()
    f2 = sb - sc
    total = (s1 - B) / 2.0 + 2.0 * f2
    n_pair = B * (B - 1) // 2
    return np.float32(total / n_pair)


_CACHED = {}


def kernel(ys: np.ndarray, labels: np.ndarray) -> np.ndarray:
    B, D = ys.shape
    C = 8
    S = 4
    key = (B, D, C, S)
    if key not in _CACHED:
        _CACHED[key] = build_gram_loss(B=B, D=D, C=C, S=S)
    nc = _CACHED[key]

    from concourse.bass_utils import run_bass_kernel_spmd

    in_maps = make_in_maps(np.asarray(ys), np.asarray(labels), B, D, C, S=S)
    res = run_bass_kernel_spmd(nc, in_maps, core_ids=list(range(C)))
    parts = [res.results[i]["out_parts"] for i in range(C)]
    return combine_parts(parts, B)


if __name__ == "__main__":
    # quick smoke: build only
    nc = build_gram_loss()
    print("built ok:", len(nc.m.functions[0].blocks), "blocks")



# revision 4
# speedup vs baseline: 6.5464x; 6.5464x over previous
"""Contrastive-loss kernel for Trainium2, 8 NeuronCores.

Math
----
reference:
    yn  = ys / clip(||ys||, 1e-6)          (row-normalize)
    cos = yn @ yn.T                         [B, B]
    pair_loss = same ? relu(2 - cos)^2 : cos^2
    loss = sum(strict_lower(pair_loss)) / (B*(B-1)/2)

Because margin M = 2 and |cos| <= 1, relu(2 - cos) == 2 - cos, so
    pair_loss = cos^2 + 4 * same * (1 - cos)
and summing the symmetric [B, B] matrix (diag cos_ii == 1):
    sum_{i>j} pair_loss = (S1 - B)/2 + 2 * (SB - SC)
with full-matrix sums
    S1 = sum cos^2        = ||yn yn^T||_F^2 = ||yn^T yn||_F^2 = ||G||_F^2
    SB = sum same         = sum_k n_k^2           (label counts)
    SC = sum same * cos   = sum_k ||sum_{i: label_i = k} yn_i||^2 = ||s||_F^2

The Frobenius identity collapses the [4096, 4096] Gram matrix to the
[2048, 2048] feature covariance G = yn^T yn, halving the matmul FLOPs,
and G is symmetric so only its lower-triangle blocks (136 of 256) are
needed: 9.1 GMAC total vs 34.4 GMAC for the row-Gram approach.

Device plan (SPMD, identical program on 8 cores):
 1. Each core loads its own 512 rows, row-normalizes (ACT square+accum,
    sqrt, DVE max/recip, scaled copy) -> yn [128, 4, 2048] bf16.
    No transpose needed: [row-part, feat] is already matmul layout for
    ynT @ yn (K = rows on partitions).
 2. s-matmul: onehot(labels)^T @ yn -> s_c [10, 2048] (PSUM f32).
 3. G-matmul: lower-triangle blocks of G_c = yn_c^T yn_c (K=512).
    PSUM tiles [128, <=512]; copies to a bf16 staging buffer ordered by
    block slot; diagonal blocks pre-scaled by 1/sqrt(2) so that the
    uniform weight-2 host sum counts them once.
 4. ONE ReduceScatter sums G_c and s_c across cores and hands each core
    a 1/8 slice: cc_in [1040, 2176] = 8 groups of (128 triangle rows +
    2 s rows); rank r gets group r. (The baseline burned ~480us in 4
    serialized AllGather calls; this is a single small-output RS.)
 5. Post-RS: square+accum the slice -> [128,1]+[2,1] partials; DMA out
    [128, 4] f32 per core; host combines (exact SB from label bincount).
"""

import os
import sys

for _p in ("/opt/trn_rl_repo", "/root/.axon_site/_ro/trn_rl_repo"):
    if _p not in sys.path and os.path.isdir(_p):
        sys.path.append(_p)

import numpy as np

import concourse.bass as bass
import concourse.mybir as mybir
import concourse.tile as tile
from concourse.bass import ds, ts  # noqa: F401

F32 = mybir.dt.float32
BF16 = mybir.dt.bfloat16
AF = mybir.ActivationFunctionType
ALU = mybir.AluOpType

P = 128   # partitions
NK = 10   # number of label classes


def _split_multi_waits(nc):
    """Split instructions carrying >1 semaphore wait.

    The walrus in this environment rejects compute instructions with more
    than one sync-wait command ("Too many sync wait commands"). Move the
    extra waits onto standalone EventSemaphore instructions inserted just
    before, on the same engine - semantically identical (the engine's
    sequencer blocks on each in order).
    """
    n_split = 0
    for fn in nc.m.functions:
        for bb in fn.blocks:
            new_insts = []
            for ins in bb.instructions:
                si = ins.sync_info
                if (
                    si is not None
                    and len(si.on_wait) > 1
                    and not isinstance(ins, mybir.InstEventSemaphore)
                ):
                    extra = list(si.on_wait[1:])
                    ins.sync_info = mybir.SyncInfo(
                        on_wait=[si.on_wait[0]], on_update=list(si.on_update)
                    )
                    for w in extra:
                        n_split += 1
                        ev = mybir.InstEventSemaphore(
                            name=f"antsplitwait_{n_split}_{ins.name}",
                            engine=ins.engine,
                            ins=[],
                            outs=[],
                            sync_info=mybir.SyncInfo(on_wait=[w], on_update=[]),
                            bass_nofuse=True,
                        )
                        new_insts.append(ev)
                new_insts.append(ins)
            bb.instructions = new_insts
    return n_split


def build_gram_loss(B=4096, D=2048, C=8, **_unused):
    """Build the SPMD bass program (one nc, run on C cores)."""
    Bs = B // C            # rows per core (512)
    RT = Bs // P           # 128-row K-chunks per core (4)
    DT = D // P            # feature tiles = G row-tiles (16)
    NBLK = DT * (DT + 1) // 2   # lower-triangle 128x128 blocks (136)
    assert NBLK % C == 0
    SPR = NBLK // C        # block slots per RS group (17)
    GR = P + 2             # rows per RS group: 128 triangle + 2 s rows
    CCW = SPR * P          # staging row width (2176)
    assert 2 * C >= NK     # s rows fit in the per-group s slots

    nc = bass.Bass(num_devices=C)

    ys_mine = nc.dram_tensor("ys_mine", [Bs, D], F32, kind="ExternalInput")
    oh_mine = nc.dram_tensor("oh_mine", [P, RT * NK], F32, kind="ExternalInput")
    out_parts = nc.dram_tensor("out_parts", [P, 4], F32, kind="ExternalOutput")

    cc_in = nc.dram_tensor("cc_in", [C * GR, CCW], BF16)
    cc_out = nc.dram_tensor("cc_out", [GR, CCW], BF16)

    with tile.TileContext(nc) as tc:
        with (
            tc.tile_pool(name="const", bufs=1) as const_pool,
            tc.tile_pool(name="big", bufs=1) as big_pool,
            tc.tile_pool(name="ysin", bufs=4) as ys_pool,
            tc.tile_pool(name="sqscr", bufs=2) as sq_pool,
            tc.tile_pool(name="small", bufs=8) as small_pool,
            tc.tile_pool(name="post", bufs=1) as post_pool,
            tc.tile_pool(name="psg", bufs=4, space="PSUM") as psg,
            tc.tile_pool(name="pss", bufs=2, space="PSUM") as pss,
        ):
            # ---------------- constants / label prep ----------------
            eps_tile = const_pool.tile([P, 1], F32)
            nc.gpsimd.memset(eps_tile[:], 1e-6)
            isq2 = const_pool.tile([P, 1], F32)
            nc.gpsimd.memset(isq2[:], 0.7071067811865476)

            oh_f = const_pool.tile([P, RT * NK], F32)
            nc.sync.dma_start(out=oh_f[:], in_=oh_mine[:, :])
            oh_b = const_pool.tile([P, RT, NK], BF16)
            nc.vector.tensor_copy(
                oh_b[:, :, :], oh_f[:, :].rearrange("p (t k) -> p t k", t=RT)
            )

            # s staging: rows 0..9 get the group sums, rest stays zero.
            # Row r < 8 lands at cc_in row r*GR+128; row 8+g at g*GR+129.
            s_sb = big_pool.tile([2 * C, CCW], BF16)
            nc.gpsimd.memset(s_sb[:], 0.0)

            # ---------------- phase A: normalize own rows ----------------
            yn = big_pool.tile([P, RT, D], BF16)
            for t in range(RT):
                ys_t = ys_pool.tile([P, D], F32)
                dmae = nc.sync if t % 2 == 0 else nc.gpsimd
                dmae.dma_start(out=ys_t[:], in_=ys_mine[ts(t, P), :])

                ssq = small_pool.tile([P, 1], F32, tag="ssq")
                sq_scr = sq_pool.tile([P, D], BF16)
                nc.scalar.activation(
                    sq_scr[:], ys_t[:], AF.Square, accum_out=ssq[:]
                )
                norm_t = small_pool.tile([P, 1], F32, tag="norm")
                nc.scalar.sqrt(norm_t[:], ssq[:])
                normc = small_pool.tile([P, 1], F32, tag="normc")
                nc.vector.tensor_tensor(normc[:], norm_t[:], eps_tile[:], ALU.max)
                r_t = small_pool.tile([P, 1], F32, tag="rt")
                nc.vector.reciprocal(r_t[:], normc[:])
                nc.vector.tensor_scalar_mul(yn[:, t, :], ys_t[:], r_t[:])

            # ---------------- phase B: s-matmul (warms up PE) ------------
            NW = D // 512  # 512-col chunks of s
            for w in range(NW):
                ps_s = pss.tile([NK, 512], F32)
                for k in range(RT):
                    nc.tensor.matmul(
                        ps_s[:],
                        lhsT=oh_b[:, k, :],
                        rhs=yn[:, k, ts(w, 512)],
                        start=(k == 0),
                        stop=(k == RT - 1),
                    )
                eng = nc.scalar if w % 2 == 0 else nc.vector
                if eng is nc.scalar:
                    nc.scalar.copy(s_sb[0:NK, ts(w, 512)], ps_s[:])
                else:
                    nc.vector.tensor_copy(s_sb[0:NK, ts(w, 512)], ps_s[:])

            # ---------------- phase C: G lower-triangle matmul -----------
            # tri slot order: block (i, j), j <= i, slot = i(i+1)/2 + j.
            # RS group g owns slots [17g, 17g+17); staged as one DMA each.
            tri = big_pool.tile([P, NBLK, P], BF16)
            n_copy = 0
            next_dma_slot = SPR  # fire group DMA when its last slot is done

            def stage_dma(g):
                dmae = nc.sync if g % 2 == 0 else nc.gpsimd
                dmae.dma_start(
                    out=cc_in[ds(g * GR, P), :],
                    in_=tri[:, ds(g * SPR, SPR), :],
                )

            # gpsimd (Pool) cannot access PSUM; copies go on ACT + DVE
            copy_engs = [nc.scalar, nc.vector]
            for i in range(DT):
                base = i * (i + 1) // 2
                for j0 in range(0, i + 1, 4):
                    w = min(4, i + 1 - j0)
                    W = w * P
                    ps = psg.tile([P, 512], F32)
                    for k in range(RT):
                        nc.tensor.matmul(
                            ps[:, 0:W],
                            lhsT=yn[:, k, ts(i, P)],
                            rhs=yn[:, k, j0 * P : j0 * P + W],
                            start=(k == 0),
                            stop=(k == RT - 1),
                        )
                    slot0 = base + j0
                    has_diag = j0 + w - 1 == i
                    nplain = w - 1 if has_diag else w
                    if nplain > 0:
                        eng = copy_engs[n_copy % 2]
                        n_copy += 1
                        dst = tri[:, ds(slot0, nplain), :]
                        src = ps[:, 0 : nplain * P]
                        if eng is nc.scalar:
                            nc.scalar.copy(dst, src)
                        else:
                            nc.vector.tensor_copy(dst, src)
                    if has_diag:
                        # diag block scaled by 1/sqrt(2): uniform weight-2
                        # host sum then counts it exactly once
                        eng = copy_engs[n_copy % 2]
                        n_copy += 1
                        dst = tri[:, slot0 + nplain, :]
                        src = ps[:, nplain * P : nplain * P + P]
                        if eng is nc.vector:
                            nc.vector.tensor_scalar_mul(dst, src, isq2[:])
                        else:
                            nc.scalar.activation(dst, src, AF.Copy, scale=isq2[:])
                    # fire any completed group staging DMAs
                    done = slot0 + w
                    while done >= next_dma_slot:
                        stage_dma(next_dma_slot // SPR - 1)
                        next_dma_slot += SPR

            # s pack: two clean strided DMAs into the per-group s rows
            cc_in_g = cc_in[:, :].rearrange("(c g) w -> c g w", g=GR)
            nc.sync.dma_start(out=cc_in_g[:, P, :], in_=s_sb[0:C, :])
            nc.gpsimd.dma_start(out=cc_in_g[:, P + 1, :], in_=s_sb[C : 2 * C, :])

            # ---------------- phase D: ReduceScatter + epilogue ----------
            nc.gpsimd.collective_compute(
                "ReduceScatter",
                ALU.add,
                replica_groups=[list(range(C))],
                ins=[cc_in[:, :]],
                outs=[cc_out[:, :]],
            )

            sl_a = post_pool.tile([P, CCW], BF16)
            nc.sync.dma_start(out=sl_a[:], in_=cc_out[ds(0, P), :])
            sl_b = post_pool.tile([2, CCW], BF16)
            nc.gpsimd.dma_start(out=sl_b[:], in_=cc_out[ds(P, 2), :])

            scr_a = post_pool.tile([P, CCW], BF16)
            racc_a = post_pool.tile([P, 1], F32)
            nc.scalar.activation(scr_a[:], sl_a[:], AF.Square, accum_out=racc_a[:])
            scr_b = post_pool.tile([2, CCW], BF16)
            racc_b = post_pool.tile([2, 1], F32)
            nc.scalar.activation(scr_b[:], sl_b[:], AF.Square, accum_out=racc_b[:])

            out_sb = post_pool.tile([P, 4], F32)
            nc.vector.memset(out_sb[:], 0.0)
            nc.scalar.copy(out_sb[:, 0:1], racc_a[:])
            nc.scalar.copy(out_sb[0:2, 1:2], racc_b[:])
            nc.sync.dma_start(out=out_parts[:, :], in_=out_sb[:])

    _split_multi_waits(nc)
    return nc


def make_in_maps(ys, labels, B, D, C, **_unused):
    """Shard host inputs into per-core input maps."""
    ys = np.ascontiguousarray(ys, dtype=np.float32)
    labels = np.asarray(labels).astype(np.int64)
    Bs = B // C
    RT = Bs // P
    in_maps = []
    for c in range(C):
        lab_c = labels[c * Bs : (c + 1) * Bs].reshape(RT, P)
        # oh[p, t*NK + k] = 1.0 iff labels[c*Bs + t*P + p] == k
        oh = (lab_c[:, :, None] == np.arange(NK)[None, None, :]).astype(np.float32)
        oh = oh.transpose(1, 0, 2).reshape(P, RT * NK)
        in_maps.append(
            {
                "ys_mine": ys[c * Bs : (c + 1) * Bs],
                "oh_mine": np.ascontiguousarray(oh),
            }
        )
    return in_maps


def combine_parts(parts_list, labels, B):
    """parts_list: per-core [128, 4] f32 partials -> scalar loss."""
    t2 = 0.0
    scp = 0.0
    for p in parts_list:
        p = np.asarray(p, dtype=np.float64)
        t2 += p[:, 0].sum()
        scp += p[0, 1] + p[1, 1]
    s1 = 2.0 * t2
    sc = scp
    n = np.bincount(np.asarray(labels).astype(np.int64), minlength=NK).astype(
        np.float64
    )
    sb = (n**2).sum()
    total = (s1 - B) / 2.0 + 2.0 * (sb - sc)
    n_pair = B * (B - 1) // 2
    return np.float32(total / n_pair)


_CACHED = {}


def kernel(ys: np.ndarray, labels: np.ndarray) -> np.ndarray:
    B, D = ys.shape
    C = 8
    key = (B, D, C)
    if key not in _CACHED:
        _CACHED[key] = build_gram_loss(B=B, D=D, C=C)
    nc = _CACHED[key]

    from concourse.bass_utils import run_bass_kernel_spmd

    in_maps = make_in_maps(np.asarray(ys), np.asarray(labels), B, D, C)
    res = run_bass_kernel_spmd(nc, in_maps, core_ids=list(range(C)))
    parts = [res.results[i]["out_parts"] for i in range(C)]
    return combine_parts(parts, labels, B)


if __name__ == "__main__":
    nc = build_gram_loss()
    print("built ok:", len(nc.m.functions[0].blocks), "blocks")
